# revision 3
# baseline (speedup 1.0000x reference)
"""DigitCaps dynamic-routing kernel for Trainium2 (8 NeuronCores, Bass/Tile).

Math (per routing iteration, reformulated to avoid materializing u_hat):
    u_hat[b,i,j,u] = sum_k W[i,j,u,k] * x[b,k,i]
    s[b,ju]  = sum_{ki} X[ki,b] * (c[i,j] * W[ki,ju])          (PE matmul, K=9216)
    v        = squash(s)  with the reference's quirky j-axis norm
    G[ki,ju] = sum_b X[b,ki] * v[b,ju]                         (PE matmul, K=64)
    b[i,j]   = sum_{k,u} W[ki,ju] * G[ki,ju]                   (DVE product+reduce)
    b is AllReduced (sum) over the 8 cores each iteration (batch mean).

Sharding: data-parallel over batch B=512 -> 64 rows per core; W replicated.

Design notes (v2, informed by perfetto/NTFF traces of the ~180us v1):
  - the ncfw collective stack runs a ~43us cold barrier (core-launch skew)
    that starts at ~21us regardless of kernel instructions and ends ~64us.
    it0 (uniform c, no exp) plus all input DMA + PE transposes execute
    entirely inside that window, so it0's b AllReduce fires the moment the
    barrier clears. v1's separate tiny warm-up AllReduce serialized ~24us
    ahead of the first real AR and is deleted.
  - it0 fires ONE AllReduce (payload ready long before the barrier ends;
    a split would just pay the ~9us mesh floor twice). it1 keeps the 5/4
    split so half A's exp/wp/s-matmuls overlap half B's flight.
  - wp = exp(b/B) * W was v1's top DVE cost (9 x 1.49us): the broadcast
    operand (stride-0 AP) forces DVE 1x mode ((N+151)/0.96 matches
    exactly). v2 fuses exp+broadcast on the ACT engine into a materialized
    eU4 = exp tiled over (k half, u), then does two fully-contiguous bf16
    tensor_muls per chunk -> 2x_1P mode, ~2x faster, and the ACT work
    pipelines one chunk ahead of the DVE.
  - Z (softmax normalizer) accumulates per-chunk ones-matmuls against a
    strided view of eU4; the 1/Z normalizer folds into one post-matmul
    multiply as before.
  - squash sqrt stays a DVE Newton rsqrt (quake seed + 1 iteration) so the
    ACT exp table never reloads (a table switch costs ~1.5us serial).
  - b-update per chunk: G packs k 2-per-PSUM-bank so bank-linear order IS
    (k,j,u); ACT evacuates to bf16, the W*G product and fold tree run in
    DVE 2x mode, final u-reduce on DVE, bf16 copy on ACT.
  - everything on the PE runs bf16 (fp32 LOW_HIGH matmuls are ~5x slower);
    measured end-to-end L2 err ~4e-3 vs the 2e-2 gate.
  - host pre-casts to bf16 and pre-builds the transposed x layout and the
    (k,j,u)-ordered W, halving DMA bytes. x's natural layout is rebuilt
    on-chip by PE transposes that also keep the PE busy through the load.
"""

import sys

sys.path.insert(0, "/opt/trn_rl_repo")

from contextlib import ExitStack

import numpy as np

B = 512
NCORES = 8
BL = B // NCORES  # 64 local batch rows
K = 8             # in_units (primary capsule dim)
IC = 1152         # in_channels (number of primary capsules)
J = 10            # num_units (output capsules)
U = 16            # unit_size
JU = J * U        # 160
NT = IC // 128    # 9 i-chunks of 128
NKT = K * NT      # 72 ki-chunks of 128
BETA = 1.45
NUM_ROUTING = 3

_CACHE = {}


def _build_nc():
    import concourse.bass as bass
    import concourse.tile as tile
    from concourse import bacc, mybir
    from concourse.masks import make_identity

    f32 = mybir.dt.float32
    bf16 = mybir.dt.bfloat16
    i32 = mybir.dt.int32
    Alu = mybir.AluOpType
    Act = mybir.ActivationFunctionType
    Ax = mybir.AxisListType

    nc = bacc.Bacc("TRN2", target_bir_lowering=False, debug=False,
                   num_devices=NCORES)

    # host-prepped bf16 inputs (see _prep below); x's natural layout is
    # rebuilt on-chip from the transposed one to halve the DMA footprint
    xs1 = nc.dram_tensor("xs1", [128, NKT, BL], bf16, kind="ExternalInput").ap()
    wk = nc.dram_tensor("wk", [128, NT * JU * K], bf16, kind="ExternalInput").ap()
    out = nc.dram_tensor("out", [BL, JU], f32, kind="ExternalOutput").ap()

    with tile.TileContext(nc) as tc, ExitStack() as ctx:
        consts = ctx.enter_context(tc.tile_pool(name="consts", bufs=1))
        small = ctx.enter_context(tc.tile_pool(name="small", bufs=2))
        scratch = ctx.enter_context(tc.tile_pool(name="scratch", bufs=8))
        psum = ctx.enter_context(tc.tile_pool(name="psum", bufs=1, space="PSUM"))
        dram = ctx.enter_context(tc.tile_pool(name="dram", bufs=1, space="DRAM"))

        # ---- persistent SBUF tensors ----
        x2b = consts.tile([BL, K * IC], bf16)        # x[b, (k i)] (G stationary)
        x1b = consts.tile([128, NKT, BL], bf16)      # x^T per ki-chunk (s stationary)
        w_kju = consts.tile([128, NT, K * JU], bf16)   # W[(i),(k,j,u)]
        wp = consts.tile([128, NT, K * JU], bf16)      # exp-scaled W (iters>0)
        ones = consts.tile([128, 128], bf16)         # Z broadcast matmul lhsT

        # one PSUM tensor = all 8 banks; everything slices into it
        pall = psum.tile([128, K, 512], f32)

        nc.vector.memset(ones, 1.0)

        # ---- ACT exp-table preload: first real Exp fires right after the
        # it0 AllReduce lands, on the critical path; load the table now. ----
        etp = consts.tile([BL, 1], f32)
        nc.vector.memset(etp, 0.0)
        nc.scalar.activation(etp, etp, Act.Exp, scale=1.0)

        # ---- loads: ~14 pieces so every DMA engine streams one in parallel
        # (per-engine rate is limited; per-engine queue is serial).
        # Triggers split across the two HWDGE queues (sync + scalar). ----
        wk_flat = w_kju.rearrange("p t f -> p (t f)")
        for k in range(K):
            nc.sync.dma_start(out=x1b[:, k * NT:(k + 1) * NT, :],
                              in_=xs1[:, k * NT:(k + 1) * NT, :])
        for c in range(6):
            nc.scalar.dma_start(out=wk_flat[:, c * 1920:(c + 1) * 1920],
                                in_=wk[:, c * 1920:(c + 1) * 1920])

        # ---- rebuild x2b = x1b^T on the PE (bf16 transposes, PSUM bitcast,
        # ACT evacuation). Doubles as the HAM warm-up: the PE stays busy
        # through the load phase while it waits for w_kju. ----
        ident = consts.tile([128, 128], bf16)
        make_identity(nc, ident)
        pbf = pall.bitcast(bf16)                    # [128, K, 1024] bf16 view

        def transpose_chunk(t, slot):
            k, t2 = divmod(t, NT)
            ps = pbf[:BL, slot, :128]               # [64, 128] bf16
            nc.tensor.transpose(ps, x1b[:, t, :], ident)
            dst = x2b[:, k * IC + t2 * 128:k * IC + t2 * 128 + 128]
            nc.scalar.copy(dst, ps)

        for t in range(NKT):
            transpose_chunk(t, t % 8)

        NT_A = 5
        ar_results = {}

        def _fire_ar(it, tag, b_slice, nt):
            cc_in = dram.tile([nt * 128, J], bf16, name=f"ccin{it}{tag}")
            cc_out = dram.tile([nt * 128, J], bf16, name=f"ccout{it}{tag}",
                               addr_space="Shared")
            cc_in_v = cc_in.rearrange("(t p) j -> p t j", p=128)
            for h, eng in ((0, nc.sync), (1, nc.scalar)):
                hs = slice(h * 64, (h + 1) * 64)
                eng.dma_start(out=cc_in_v[hs], in_=b_slice[hs])
            nc.gpsimd.collective_compute(
                "AllReduce", Alu.add,
                replica_groups=[list(range(NCORES))],
                ins=[cc_in[:, :]], outs=[cc_out[:, :]])
            bf_t = small.tile([128, nt, J], bf16, name=f"bf{it}{tag}")
            cc_out_v = cc_out.rearrange("(t p) j -> p t j", p=128)
            for h, eng in ((0, nc.sync), (1, nc.scalar)):
                hs = slice(h * 64, (h + 1) * 64)
                eng.dma_start(out=bf_t[hs], in_=cc_out_v[hs])
            ar_results[(it, tag)] = bf_t

        for it in range(NUM_ROUTING):
            # ---- wp = exp(b_sum/B) * w_kju (iters>0). Iteration 0 has
            # uniform c = 1/IC folded into the squash scales, so the matmul
            # rhs is just w_kju directly.
            # Per chunk: ACT fuses exp + (k-half, u) broadcast into a
            # materialized eU4 [128, 4*JU]; the two wp halves are then
            # fully-contiguous bf16 tensor_muls -> DVE 2x mode. The Z
            # ones-matmuls accumulate from a strided eU4 view. ----
            if it > 0:
                if it == 1:
                    bf_parts = [(ar_results[(0, "s")], 0)]
                else:
                    bf_parts = [(ar_results[(1, "a")], 0),
                                (ar_results[(1, "b")], NT_A)]

                def bf_of(t2):
                    for tile_, base in reversed(bf_parts):
                        if t2 >= base:
                            return tile_[:, t2 - base, :]
                    raise AssertionError

                eus = []
                for t2 in range(NT):
                    eu = scratch.tile([128, 4, J, U], bf16,
                                      name=f"eu{it}", bufs=3)
                    nc.scalar.activation(
                        eu, bf_of(t2).unsqueeze(1).unsqueeze(-1)
                        .broadcast_to([128, 4, J, U]),
                        Act.Exp, scale=1.0 / B)
                    eus.append(eu)
                    euf = eu.rearrange("p h j u -> p (h j u)")
                    for h in range(2):
                        nc.vector.tensor_mul(
                            wp[:, t2, h * 4 * JU:(h + 1) * 4 * JU],
                            w_kju[:, t2, h * 4 * JU:(h + 1) * 4 * JU],
                            euf)
                rhs_src = wp
            else:
                rhs_src = w_kju

            # ---- s = X1^T @ wp : accumulate 72 chunks into PSUM bank 0.
            # The per-chunk Z matmul (Z[j] = sum_i exp(b[i,j])) rides the
            # PE queue right behind each chunk's s-matmuls; zinv computes
            # on the DVE while the last s-matmuls still stream. ----
            sp = pall[:BL, 0, :JU]
            zinv = None
            for t2 in range(NT):
                for k in range(K):
                    t = k * NT + t2
                    first = (t2 == 0 and k == 0)
                    last = (t2 == NT - 1 and k == K - 1)
                    nc.tensor.matmul(sp, x1b[:, t, :],
                                     rhs_src[:, t2, k * JU:(k + 1) * JU],
                                     start=first, stop=last)
                if it > 0:
                    zp = pall[:, 1, :J]
                    nc.tensor.matmul(zp, ones, eus[t2][:, 0, :, 0],
                                     start=(t2 == 0), stop=(t2 == NT - 1))
                    if t2 == NT - 1:
                        zinv = small.tile([BL, J], f32, name=f"zinv{it}")
                        nc.vector.reciprocal(zinv, zp[:BL, :])

            if it > 0:
                # s_norm = s * (1/Z_j), also evacuates PSUM
                s_sb = small.tile([BL, JU], f32, name=f"s_sb{it}")
                nc.vector.tensor_mul(
                    s_sb.rearrange("b (j u) -> b j u", j=J),
                    sp.rearrange("b (j u) -> b j u", j=J),
                    zinv.unsqueeze(-1).broadcast_to([BL, J, U]))
            else:
                s_sb = small.tile([BL, JU], f32, name=f"s_sb{it}")
                nc.vector.tensor_copy(s_sb, sp)

            # ---- squash (reference quirk: norm over the j axis per (b,u)).
            # All on DVE; sqrt via quake-seed Newton rsqrt (no ACT tables). ----
            ssq = small.tile([BL, JU], f32, name=f"ssq{it}")
            nc.vector.tensor_mul(ssq, s_sb, s_sb)
            msq = small.tile([BL, U], f32, name=f"msq{it}")
            nc.vector.tensor_reduce(
                msq, ssq.rearrange("b (j u) -> b u j", j=J),
                axis=Ax.X, op=Alu.add)
            # iteration 0: s here is actually IC*s, so scale msq by 1/IC^2
            # and fold 1/IC into the final v multiply
            sc2 = 1.0 / (IC * IC) if it == 0 else 1.0
            scv = 1.0 / (IC * IC) if it == 0 else 1.0
            # y ~= rsqrt(msq): quake seed + 1 Newton iteration
            ti = small.tile([BL, U], i32, name=f"ti{it}")
            nc.vector.tensor_scalar(ti, msq.bitcast(i32), 1, 0,
                                    op0=Alu.arith_shift_right,
                                    op1=Alu.logical_shift_left)
            y0i = small.tile([BL, U], i32, name=f"y0i{it}")
            nc.vector.tensor_scalar(y0i, ti, 0x5f3759df, -1,
                                    op0=Alu.subtract, op1=Alu.mult)
            y0 = y0i.bitcast(f32)
            yc = y0
            for n in range(1):
                t_a = small.tile([BL, U], f32, name=f"na{it}_{n}")
                nc.vector.tensor_mul(t_a, yc, yc)
                nc.vector.scalar_tensor_tensor(
                    out=t_a, in0=t_a, scalar=0.5, in1=msq,
                    op0=Alu.mult, op1=Alu.mult)
                nc.vector.tensor_scalar(t_a, t_a, -1.0, 1.5,
                                        op0=Alu.mult, op1=Alu.add)
                t_b = small.tile([BL, U], f32, name=f"nb{it}_{n}")
                nc.vector.tensor_mul(t_b, yc, t_a)
                yc = t_b
            # f = msq*y * 1/(beta + msq*sc2) (scaled for it0)
            tpb = small.tile([BL, U], f32, name=f"tpb{it}")
            nc.vector.tensor_scalar(tpb, msq, sc2, BETA,
                                    op0=Alu.mult, op1=Alu.add)
            rin = small.tile([BL, U], f32, name=f"rin{it}")
            nc.vector.reciprocal(rin, tpb)
            fv = small.tile([BL, U], f32, name=f"fv{it}")
            nc.vector.tensor_mul(fv, msq, yc)
            nc.vector.tensor_mul(fv, fv, rin)
            v = small.tile([BL, JU], f32, name=f"v{it}")
            nc.vector.scalar_tensor_tensor(
                out=v.rearrange("b (j u) -> b j u", j=J),
                in0=s_sb.rearrange("b (j u) -> b j u", j=J),
                scalar=scv,
                in1=fv.unsqueeze(1).broadcast_to([BL, J, U]),
                op0=Alu.mult, op1=Alu.mult)

            if it == NUM_ROUTING - 1:
                nc.sync.dma_start(out=out[:, :JU // 2], in_=v[:, :JU // 2])
                nc.scalar.dma_start(out=out[:, JU // 2:], in_=v[:, JU // 2:])
                continue
            vb = small.tile([BL, JU], bf16, name=f"vb{it}")
            nc.scalar.copy(vb, v)

            # ---- G = X2^T-chunks @ v per t2; banks ping-pong in halves
            # (even t2 -> banks 0-3, odd -> 4-7; k packed 2-per-bank).
            # b_part[:, t2, j] = sum_{u,k} W * G via product + X-reduce. ----
            b_part = small.tile([128, NT, J], f32, name=f"bpart{it}")
            b_bf = small.tile([128, NT, J], bf16, name=f"bbf{it}")
            for t2 in range(NT):
                b0 = 0 if t2 % 2 == 0 else 4
                for k in range(K):
                    bank = b0 + k // 2
                    kk = k % 2
                    nc.tensor.matmul(
                        pall[:, bank, kk * JU:(kk + 1) * JU],
                        x2b[:, (k * IC + t2 * 128):(k * IC + t2 * 128) + 128],
                        vb, start=True, stop=True)
                # product P[(j,u,k)] = W * G, G read straight out of PSUM
                # via a 4D AP [j, u, bank, kk]
                # The PSUM bank layout (bank, kk, ju) read in linear order IS
                # (k, j, u) order -> contiguous evacuation, contiguous product
                # against w_kju (2x DVE mode), and the k-reduction becomes
                # contiguous TT-add folds (2x) instead of a 1x strided reduce.
                g5 = scratch.tile([128, JU * K], bf16, name="g5", bufs=3)
                nc.scalar.copy(g5.rearrange("p (b f) -> p b f", b=4),
                               pall[:, b0:b0 + 4, :2 * JU])
                prod = scratch.tile([128, JU * K], bf16, name="prod", bufs=3)
                nc.vector.tensor_mul(prod, w_kju[:, t2, :], g5)
                p3 = prod.rearrange("p (k f) -> p k f", k=K)
                f4 = scratch.tile([128, 4, JU], bf16, name="f4", bufs=3)
                nc.vector.tensor_add(f4, p3[:, :4], p3[:, 4:])
                f2 = scratch.tile([128, 2, JU], bf16, name="f2", bufs=3)
                nc.vector.tensor_add(f2, f4[:, :2], f4[:, 2:])
                f1 = scratch.tile([128, JU], bf16, name="f1", bufs=3)
                nc.vector.tensor_add(f1, f2[:, 0], f2[:, 1])
                nc.vector.tensor_reduce(
                    b_part[:, t2, :],
                    f1.rearrange("p (j u) -> p j u", j=J),
                    axis=Ax.X, op=Alu.add)
                nc.scalar.copy(b_bf[:, t2, :], b_part[:, t2, :])
                if it == 1 and t2 == NT_A - 1:
                    # ---- it1 AllReduce half A as soon as its chunks are
                    # done so it overlaps the b-update tail of half B, and
                    # it2's exp/wp/s for half A overlaps AllReduce B ----
                    _fire_ar(it, "a", b_bf[:, :NT_A, :], NT_A)
            if it == 0:
                # it0's payload is ready long before the ncfw barrier
                # clears; one AR avoids paying the mesh floor twice.
                _fire_ar(it, "s", b_bf, NT)
            else:
                _fire_ar(it, "b", b_bf[:, NT_A:, :], NT - NT_A)

    nc.compile()
    return nc


def _prep(x, W):
    """Host-side prep: bf16 cast + device layouts for x and W."""
    import ml_dtypes

    bf16 = ml_dtypes.bfloat16
    x = np.asarray(x, dtype=np.float32)
    W = np.asarray(W, dtype=np.float32)
    xb = x.astype(bf16)                      # (B, K, IC)
    # W (k,j,u): [p, (t2, k j u)]
    wk = np.ascontiguousarray(
        W.reshape(NT, 128, J, U, K).transpose(1, 0, 4, 2, 3)
        .reshape(128, NT * K * J * U).astype(bf16))
    in_maps = []
    for c in range(NCORES):
        rows = xb[c * BL:(c + 1) * BL]       # (BL, K, IC)
        xs1 = np.ascontiguousarray(
            rows.reshape(BL, K, NT, 128).transpose(3, 1, 2, 0)
            .reshape(128, NKT, BL))
        in_maps.append({
            "xs1": xs1,
            "wk": wk,
        })
    return in_maps


def _run(x, W, trace=False, **kw):
    from concourse import bass_utils

    nc = _get_nc()
    in_maps = _prep(x, W)
    res = bass_utils.run_bass_kernel_spmd(
        nc, in_maps, core_ids=list(range(NCORES)), trace=trace, **kw)
    outs = [res.results[c]["out"] for c in range(NCORES)]
    full = np.concatenate(outs, axis=0).reshape(B, J, 4, U // 4)
    return full, res


def _get_nc():
    if "nc" not in _CACHE:
        _CACHE["nc"] = _build_nc()
    return _CACHE["nc"]


def kernel(x, W):
    full, _ = _run(x, W, trace=False)
    return full


# revision 4
# speedup vs baseline: 1.0554x; 1.0554x over previous
"""DigitCaps dynamic-routing kernel for Trainium2 (8 NeuronCores, Bass/Tile).

Math (per routing iteration, reformulated to avoid materializing u_hat):
    u_hat[b,i,j,u] = sum_k W[i,j,u,k] * x[b,k,i]
    s[b,ju]  = sum_{ki} X[ki,b] * (c[i,j] * W[ki,ju])          (PE matmul, K=9216)
    v        = squash(s)  with the reference's quirky j-axis norm
    G[ki,ju] = sum_b X[b,ki] * v[b,ju]                         (PE matmul, K=64)
    b[i,j]   = sum_{k,u} W[ki,ju] * G[ki,ju]                   (DVE product+reduce)
    b is summed over the 8 cores each routing iteration (batch mean).

Sharding: data-parallel over batch B=512 -> 64 rows per core; W replicated.

Design notes (v3, informed by perfetto/NTFF traces of the 180us v1/183us v2):
  - the ncfw collective stack runs a ~35-43us cold barrier (core-launch skew)
    that starts at ~21us regardless of kernel instructions, plus a fixed
    ~11us first-collective wakeup after it.  it0 (uniform c, no exp) plus
    all input DMA and the PE transposes execute inside that window, so
    it0's collective fires the moment the barrier clears.  A separate
    warm-up collective does NOT help: the wakeup+cold cost is paid
    serially after the barrier either way (measured both ways).
  - the cross-core b exchange is an AllGather + local DVE fold sum, not an
    AllReduce: the 8-core mesh AR floor is ~10-24us while AG is ~5us, and
    the 23KB payload makes everything floor-dominated.  b halves are
    PE-transposed to [nJ, 128] so the AG stacks ranks on the partition
    axis and ONE hardware DMA-transpose brings back [128, (rank, t2, j)]
    with clean descriptors; 3 bf16 fold adds reproduce the sum.
  - collective doorbells pay ~6us of DMA-completion-semaphore latency
    after the staging write lands in HBM; the it1 exchange is split 4/5 so
    half A's doorbell fires mid b-update and half B's flight overlaps
    it2's half-A exp/wp/s-matmuls.
  - wp = exp(b_sum/B) * W runs with the exp+broadcast fused on the ACT
    engine into a materialized eU4 (k-half, u replica), so the two wp
    tensor_muls per chunk are fully contiguous bf16 -> DVE 2x mode
    (measured 488ns vs 1490ns for the v1 broadcast-AP form).
  - b-update DVE work is processed two i-chunks per instruction (one
    product + 3 fold adds + one reduce per pair), amortizing the ~60ns
    DVE instruction overheads; PSUM keeps the 4-bank ping-pong.
  - input DMA uses all three DMA rings (sync: x, scalar+gpsimd: W halves)
    so the ~4.1MB load finishes ~12us earlier; it0's s-matmul chain was
    gating the it0 doorbell in v2.
  - everything on the PE runs bf16 (fp32 LOW_HIGH matmuls are ~5x slower);
    measured end-to-end L2 err ~4e-3 vs the 2e-2 gate.
  - squash sqrt is a DVE Newton rsqrt (quake seed + 1 iteration) so the
    ACT exp table never reloads (a table switch costs ~1.5us serial).
"""

import sys

sys.path.insert(0, "/opt/trn_rl_repo")

from contextlib import ExitStack

import numpy as np

B = 512
NCORES = 8
BL = B // NCORES  # 64 local batch rows
K = 8             # in_units (primary capsule dim)
IC = 1152         # in_channels (number of primary capsules)
J = 10            # num_units (output capsules)
U = 16            # unit_size
JU = J * U        # 160
NT = IC // 128    # 9 i-chunks of 128
NKT = K * NT      # 72 ki-chunks of 128
BETA = 1.45
NUM_ROUTING = 3

_CACHE = {}


def _build_nc():
    import concourse.bass as bass
    import concourse.tile as tile
    from concourse import bacc, mybir
    from concourse.masks import make_identity

    f32 = mybir.dt.float32
    bf16 = mybir.dt.bfloat16
    i32 = mybir.dt.int32
    Alu = mybir.AluOpType
    Act = mybir.ActivationFunctionType
    Ax = mybir.AxisListType

    nc = bacc.Bacc("TRN2", target_bir_lowering=False, debug=False,
                   num_devices=NCORES)

    # host-prepped bf16 inputs (see _prep below); x's natural layout is
    # rebuilt on-chip from the transposed one to halve the DMA footprint
    xs1 = nc.dram_tensor("xs1", [128, NKT, BL], bf16, kind="ExternalInput").ap()
    wk = nc.dram_tensor("wk", [128, NT * JU * K], bf16, kind="ExternalInput").ap()
    out = nc.dram_tensor("out", [BL, JU], f32, kind="ExternalOutput").ap()

    with tile.TileContext(nc) as tc, ExitStack() as ctx:
        consts = ctx.enter_context(tc.tile_pool(name="consts", bufs=1))
        small = ctx.enter_context(tc.tile_pool(name="small", bufs=2))
        scratch = ctx.enter_context(tc.tile_pool(name="scratch", bufs=8))
        psum = ctx.enter_context(tc.tile_pool(name="psum", bufs=1, space="PSUM"))
        dram = ctx.enter_context(tc.tile_pool(name="dram", bufs=1, space="DRAM"))

        # ---- persistent SBUF tensors ----
        x2b = consts.tile([BL, K * IC], bf16)        # x[b, (k i)] (G stationary)
        x1b = consts.tile([128, NKT, BL], bf16)      # x^T per ki-chunk (s stationary)
        w_kju = consts.tile([128, NT, K * JU], bf16)   # W[(i),(k,j,u)]
        wp = consts.tile([128, NT, K * JU], bf16)      # exp-scaled W (iters>0)
        ones = consts.tile([128, 128], bf16)         # Z broadcast matmul lhsT

        # one PSUM tensor = all 8 banks; everything slices into it
        pall = psum.tile([128, K, 512], f32)

        nc.vector.memset(ones, 1.0)

        # ---- ACT exp-table preload: the first real Exp fires right after
        # the it0 exchange lands, on the critical path; load the table now.
        etp = consts.tile([BL, 1], f32)
        nc.vector.memset(etp, 0.0)
        nc.scalar.activation(etp, etp, Act.Exp, scale=1.0)

        # ---- loads on all three DMA rings: x (1.18MB) on the sync HWDGE
        # ring, W (2.95MB) split between the scalar HWDGE ring and the
        # gpsimd SWDGE ring, interleaved so W arrives roughly in t2 order.
        wk_flat = w_kju.rearrange("p t f -> p (t f)")
        for k in range(K):
            nc.sync.dma_start(out=x1b[:, k * NT:(k + 1) * NT, :],
                              in_=xs1[:, k * NT:(k + 1) * NT, :])
        for c in range(6):
            eng = nc.scalar if c % 2 == 0 else nc.gpsimd
            eng.dma_start(out=wk_flat[:, c * 1920:(c + 1) * 1920],
                          in_=wk[:, c * 1920:(c + 1) * 1920])

        # ---- rebuild x2b = x1b^T on the PE (bf16 transposes, PSUM bitcast,
        # ACT evacuation). Doubles as the HAM warm-up: the PE stays busy
        # through the load phase while it waits for w_kju. ----
        ident = consts.tile([128, 128], bf16)
        make_identity(nc, ident)
        pbf = pall.bitcast(bf16)                    # [128, K, 1024] bf16 view

        def transpose_chunk(t, slot):
            k, t2 = divmod(t, NT)
            ps = pbf[:BL, slot, :128]               # [64, 128] bf16
            nc.tensor.transpose(ps, x1b[:, t, :], ident)
            dst = x2b[:, k * IC + t2 * 128:k * IC + t2 * 128 + 128]
            nc.scalar.copy(dst, ps)

        for t in range(NKT):
            transpose_chunk(t, t % 8)

        NT_A = 4
        ar_results = {}

        def _fire_ag(it, tag, b_bf, base, nt):
            """Cross-core b sum: PE-transpose the half, AllGather rank-major
            on the partition axis, DMA-transpose back, fold 8->1 on DVE."""
            nJ = nt * J
            tp = pbf[:nJ, 7, 640:768]               # free PSUM columns
            nc.tensor.transpose(
                tp, b_bf[:, base:base + nt, :].rearrange("p t j -> p (t j)"),
                ident)
            ct = small.tile([nJ, 128], bf16, name=f"ct{it}{tag}")
            nc.scalar.copy(ct, tp)
            cc_in = dram.tile([nJ, 128], bf16, name=f"ccin{it}{tag}")
            cc_out = dram.tile([NCORES * nJ, 128], bf16,
                               name=f"ccout{it}{tag}", addr_space="Shared")
            eng = nc.scalar if tag == "b" else nc.sync
            eng.dma_start(out=cc_in, in_=ct)
            nc.gpsimd.collective_compute(
                "AllGather", Alu.bypass,
                replica_groups=[list(range(NCORES))],
                ins=[cc_in[:, :]], outs=[cc_out[:, :]])
            rg = small.tile([128, NCORES, nJ], bf16, name=f"rg{it}{tag}")
            eng.dma_start_transpose(out=rg.rearrange("p r f -> p (r f)"),
                                    in_=cc_out[:, :])
            g4 = small.tile([128, 4, nJ], bf16, name=f"agf4{it}{tag}")
            nc.vector.tensor_add(g4, rg[:, :4], rg[:, 4:])
            g2 = small.tile([128, 2, nJ], bf16, name=f"agf2{it}{tag}")
            nc.vector.tensor_add(g2, g4[:, :2], g4[:, 2:])
            bt = small.tile([128, nt, J], bf16, name=f"bf{it}{tag}")
            nc.vector.tensor_add(bt.rearrange("p t j -> p (t j)"),
                                 g2[:, 0], g2[:, 1])
            ar_results[(it, tag)] = bt

        for it in range(NUM_ROUTING):
            # ---- wp = exp(b_sum/B) * w_kju (iters>0). Iteration 0 has
            # uniform c = 1/IC folded into the squash scales, so the matmul
            # rhs is just w_kju directly.
            # Per chunk: ACT fuses exp + (k-half, u) broadcast into a
            # materialized eU4 [128, 4*JU]; the two wp halves are then
            # fully-contiguous bf16 tensor_muls -> DVE 2x mode. The Z
            # ones-matmuls accumulate from a strided eU4 view. ----
            if it > 0:
                if it == 1:
                    bf_parts = [(ar_results[(0, "s")], 0)]
                else:
                    bf_parts = [(ar_results[(1, "a")], 0),
                                (ar_results[(1, "b")], NT_A)]

                def bf_of(t2):
                    for tile_, bs in reversed(bf_parts):
                        if t2 >= bs:
                            return tile_[:, t2 - bs, :]
                    raise AssertionError

                eus = []
                for t2 in range(NT):
                    eu = scratch.tile([128, 4, J, U], bf16,
                                      name=f"eu{it}", bufs=3)
                    nc.scalar.activation(
                        eu, bf_of(t2).unsqueeze(1).unsqueeze(-1)
                        .broadcast_to([128, 4, J, U]),
                        Act.Exp, scale=1.0 / B)
                    eus.append(eu)
                    euf = eu.rearrange("p h j u -> p (h j u)")
                    for h in range(2):
                        nc.vector.tensor_mul(
                            wp[:, t2, h * 4 * JU:(h + 1) * 4 * JU],
                            w_kju[:, t2, h * 4 * JU:(h + 1) * 4 * JU],
                            euf)
                rhs_src = wp
            else:
                rhs_src = w_kju

            # ---- s = X1^T @ wp : accumulate 72 chunks into PSUM bank 0.
            # The per-chunk Z matmul (Z[j] = sum_i exp(b[i,j])) rides the
            # PE queue right behind each chunk's s-matmuls; zinv computes
            # on the DVE while the last s-matmuls still stream. ----
            sp = pall[:BL, 0, :JU]
            zinv = None
            for t2 in range(NT):
                for k in range(K):
                    t = k * NT + t2
                    first = (t2 == 0 and k == 0)
                    last = (t2 == NT - 1 and k == K - 1)
                    nc.tensor.matmul(sp, x1b[:, t, :],
                                     rhs_src[:, t2, k * JU:(k + 1) * JU],
                                     start=first, stop=last)
                if it > 0:
                    zp = pall[:, 1, :J]
                    nc.tensor.matmul(zp, ones, eus[t2][:, 0, :, 0],
                                     start=(t2 == 0), stop=(t2 == NT - 1))
                    if t2 == NT - 1:
                        zinv = small.tile([BL, J], f32, name=f"zinv{it}")
                        nc.vector.reciprocal(zinv, zp[:BL, :])

            if it > 0:
                # s_norm = s * (1/Z_j), also evacuates PSUM
                s_sb = small.tile([BL, JU], f32, name=f"s_sb{it}")
                nc.vector.tensor_mul(
                    s_sb.rearrange("b (j u) -> b j u", j=J),
                    sp.rearrange("b (j u) -> b j u", j=J),
                    zinv.unsqueeze(-1).broadcast_to([BL, J, U]))
            else:
                s_sb = small.tile([BL, JU], f32, name=f"s_sb{it}")
                nc.vector.tensor_copy(s_sb, sp)

            # ---- squash (reference quirk: norm over the j axis per (b,u)).
            # All on DVE; sqrt via quake-seed Newton rsqrt (no ACT tables). ----
            ssq = small.tile([BL, JU], f32, name=f"ssq{it}")
            nc.vector.tensor_mul(ssq, s_sb, s_sb)
            msq = small.tile([BL, U], f32, name=f"msq{it}")
            nc.vector.tensor_reduce(
                msq, ssq.rearrange("b (j u) -> b u j", j=J),
                axis=Ax.X, op=Alu.add)
            # iteration 0: s here is actually IC*s, so scale msq by 1/IC^2
            # and fold 1/IC into the final v multiply
            sc2 = 1.0 / (IC * IC) if it == 0 else 1.0
            scv = 1.0 / (IC * IC) if it == 0 else 1.0
            # y ~= rsqrt(msq): quake seed + 1 Newton iteration
            ti = small.tile([BL, U], i32, name=f"ti{it}")
            nc.vector.tensor_scalar(ti, msq.bitcast(i32), 1, 0,
                                    op0=Alu.arith_shift_right,
                                    op1=Alu.logical_shift_left)
            y0i = small.tile([BL, U], i32, name=f"y0i{it}")
            nc.vector.tensor_scalar(y0i, ti, 0x5f3759df, -1,
                                    op0=Alu.subtract, op1=Alu.mult)
            y0 = y0i.bitcast(f32)
            yc = y0
            for n in range(1):
                t_a = small.tile([BL, U], f32, name=f"na{it}_{n}")
                nc.vector.tensor_mul(t_a, yc, yc)
                nc.vector.scalar_tensor_tensor(
                    out=t_a, in0=t_a, scalar=0.5, in1=msq,
                    op0=Alu.mult, op1=Alu.mult)
                nc.vector.tensor_scalar(t_a, t_a, -1.0, 1.5,
                                        op0=Alu.mult, op1=Alu.add)
                t_b = small.tile([BL, U], f32, name=f"nb{it}_{n}")
                nc.vector.tensor_mul(t_b, yc, t_a)
                yc = t_b
            # f = msq*y * 1/(beta + msq*sc2) (scaled for it0)
            tpb = small.tile([BL, U], f32, name=f"tpb{it}")
            nc.vector.tensor_scalar(tpb, msq, sc2, BETA,
                                    op0=Alu.mult, op1=Alu.add)
            rin = small.tile([BL, U], f32, name=f"rin{it}")
            nc.vector.reciprocal(rin, tpb)
            fv = small.tile([BL, U], f32, name=f"fv{it}")
            nc.vector.tensor_mul(fv, msq, yc)
            nc.vector.tensor_mul(fv, fv, rin)
            v = small.tile([BL, JU], f32, name=f"v{it}")
            nc.vector.scalar_tensor_tensor(
                out=v.rearrange("b (j u) -> b j u", j=J),
                in0=s_sb.rearrange("b (j u) -> b j u", j=J),
                scalar=scv,
                in1=fv.unsqueeze(1).broadcast_to([BL, J, U]),
                op0=Alu.mult, op1=Alu.mult)

            if it == NUM_ROUTING - 1:
                nc.sync.dma_start(out=out[:, :JU // 2], in_=v[:, :JU // 2])
                nc.scalar.dma_start(out=out[:, JU // 2:], in_=v[:, JU // 2:])
                continue
            vb = small.tile([BL, JU], bf16, name=f"vb{it}")
            nc.scalar.copy(vb, v)

            # ---- G = X2^T-chunks @ v per t2; banks ping-pong in halves
            # (even t2 -> banks 0-3, odd -> 4-7; k packed 2-per-bank).
            # The PSUM bank-linear order IS (k,j,u) -> contiguous ACT
            # evacuation and contiguous DVE product against w_kju.
            # DVE work runs two chunks per instruction (pairs) to amortize
            # instruction overheads: product, 3 fold adds, one u-reduce. ----
            b_part = small.tile([128, NT, J], f32, name=f"bpart{it}")
            b_bf = small.tile([128, NT, J], bf16, name=f"bbf{it}")
            g5 = None

            def _pair_dve(lo, n):
                prod = scratch.tile([128, 2, JU * K], bf16, name="prod",
                                    bufs=2)
                pr = prod[:, :n, :]
                nc.vector.tensor_mul(pr, w_kju[:, lo:lo + n, :],
                                     g5[:, :n, :])
                p3 = pr.rearrange("p c (k f) -> p c k f", k=K)
                f4 = scratch.tile([128, 2, 4, JU], bf16, name="bf4", bufs=2)
                nc.vector.tensor_add(f4[:, :n], p3[:, :, :4], p3[:, :, 4:])
                f2 = scratch.tile([128, 2, 2, JU], bf16, name="bf2", bufs=2)
                nc.vector.tensor_add(f2[:, :n], f4[:, :n, :2], f4[:, :n, 2:])
                f1 = scratch.tile([128, 2, JU], bf16, name="bf1", bufs=2)
                nc.vector.tensor_add(f1[:, :n], f2[:, :n, 0], f2[:, :n, 1])
                nc.vector.tensor_reduce(
                    b_part[:, lo:lo + n, :],
                    f1[:, :n].rearrange("p c (j u) -> p c j u", j=J),
                    axis=Ax.X, op=Alu.add)
                nc.scalar.copy(b_bf[:, lo:lo + n, :], b_part[:, lo:lo + n, :])

            for t2 in range(NT):
                b0 = 0 if t2 % 2 == 0 else 4
                if t2 % 2 == 0:
                    g5 = scratch.tile([128, 2, JU * K], bf16, name="g5",
                                      bufs=2)
                for k in range(K):
                    bank = b0 + k // 2
                    kk = k % 2
                    nc.tensor.matmul(
                        pall[:, bank, kk * JU:(kk + 1) * JU],
                        x2b[:, (k * IC + t2 * 128):(k * IC + t2 * 128) + 128],
                        vb, start=True, stop=True)
                nc.scalar.copy(
                    g5[:, t2 % 2, :].rearrange("p (b f) -> p b f", b=4),
                    pall[:, b0:b0 + 4, :2 * JU])
                if t2 % 2 == 1:
                    _pair_dve(t2 - 1, 2)
                if it == 1 and t2 == NT_A:
                    # half A's doorbell fires mid b-update; its flight
                    # overlaps the B half, and it2's half-A exp/wp/s
                    # overlaps half B's flight.
                    _fire_ag(it, "a", b_bf, 0, NT_A)
            _pair_dve(NT - 1, 1)
            if it == 0:
                # it0's payload is ready while the ncfw barrier is still
                # settling; a single exchange avoids a second floor.
                _fire_ag(it, "s", b_bf, 0, NT)
            else:
                _fire_ag(it, "b", b_bf, NT_A, NT - NT_A)

    nc.compile()
    return nc


def _prep(x, W):
    """Host-side prep: bf16 cast + device layouts for x and W."""
    import ml_dtypes

    bf16 = ml_dtypes.bfloat16
    x = np.asarray(x, dtype=np.float32)
    W = np.asarray(W, dtype=np.float32)
    xb = x.astype(bf16)                      # (B, K, IC)
    # W (k,j,u): [p, (t2, k j u)]
    wk = np.ascontiguousarray(
        W.reshape(NT, 128, J, U, K).transpose(1, 0, 4, 2, 3)
        .reshape(128, NT * K * J * U).astype(bf16))
    in_maps = []
    for c in range(NCORES):
        rows = xb[c * BL:(c + 1) * BL]       # (BL, K, IC)
        xs1 = np.ascontiguousarray(
            rows.reshape(BL, K, NT, 128).transpose(3, 1, 2, 0)
            .reshape(128, NKT, BL))
        in_maps.append({
            "xs1": xs1,
            "wk": wk,
        })
    return in_maps


def _run(x, W, trace=False, **kw):
    from concourse import bass_utils

    nc = _get_nc()
    in_maps = _prep(x, W)
    res = bass_utils.run_bass_kernel_spmd(
        nc, in_maps, core_ids=list(range(NCORES)), trace=trace, **kw)
    outs = [res.results[c]["out"] for c in range(NCORES)]
    full = np.concatenate(outs, axis=0).reshape(B, J, 4, U // 4)
    return full, res


def _get_nc():
    if "nc" not in _CACHE:
        _CACHE["nc"] = _build_nc()
    return _CACHE["nc"]


def kernel(x, W):
    full, _ = _run(x, W, trace=False)
    return full
